# revision 19
# baseline (speedup 1.0000x reference)
"""Multi-head causal attention (B=1, T=4096, D=768, H=12) on 8 trn2 cores.

Sharding: 16 uniform head-slots (2 per core), 12 real heads + 4 dummy
(zero-weight) slots.  Every core runs the IDENTICAL program (SPMD); cores
differ only in the weight data they receive.  Each core computes, for its
two head-slots, the full causal attention over all 4096 tokens plus that
slot-pair's partial output projection (out.T = wo_slice.T @ headout).  The
host sums the 8 partial [768, 4096] fp16 outputs, transposes, adds bias.

Pipeline structure (v4): x arrives in 8 token blocks (block-major DRAM
layout, one 128-line DMA per block; all constants are host-packed so each
needs a single DMA).  Per block tt: project Q/K for those 512 tokens,
build V^T directly via transposed matmuls (lhsT = x chunk), then run
attention for query tile tt.  Softmax normalization is deferred and staged
across later emission points (denominator bounce -> batched reciprocal ->
partition broadcast -> scale + out-projection); the last two query tiles
get their own single-tile rounds, partially emitted between qtile-7
attention groups, to shrink the serial tail.
"""

import math
import numpy as np
import ml_dtypes
from contextlib import ExitStack

import concourse.bass as bass
import concourse.bacc as bacc
import concourse.mybir as mybir
import concourse.tile as tile
from concourse.bass_utils import run_bass_kernel_spmd

BF16 = mybir.dt.bfloat16
F16 = mybir.dt.float16
F32 = mybir.dt.float32
AF = mybir.ActivationFunctionType

T = 4096
D_MODEL = 768
HEAD_DIM = 64
N_HEADS = 12
N_CORES = 8
QT = 512                  # query tile width (one full PSUM bank per chunk)
KC = 128                  # key chunk (psum partition dim)
GRP = 3                   # score chunk-jobs per exp group -> ACT free dim 1536
NQT = T // QT             # 8 query tiles / token blocks
CCH = D_MODEL // 128      # 6 contraction chunks
BLK = QT * CCH            # 3072 cols per block in xTb layout
VST = 208                 # V2 column stride per key chunk

_PROGRAM_CACHE = {}


def build_program():
    nc = bacc.Bacc(None)

    # xTb block-major: xTb[p, tt*3072 + j*512 + i] = x[tt*512 + i, j*128 + p]
    xT_d = nc.declare_dram_parameter("xTb", [128, NQT * BLK], BF16, isOutput=False)
    # w2 pre-packed in SBUF layout: w2[p, (s*6+j)*128 + d] = W_s[j*128+p, d]
    w_d = nc.declare_dram_parameter("w2", [128, 3 * CCH * 128], BF16, isOutput=False)
    b_d = nc.declare_dram_parameter("bqkv", [128, 3], F32, isOutput=False)
    bvr_d = nc.declare_dram_parameter("bvrow", [1, 128], BF16, isOutput=False)
    wo_d = nc.declare_dram_parameter("wo2", [128, D_MODEL], BF16, isOutput=False)
    mk_d = nc.declare_dram_parameter("masks2", [128, 4 * QT], BF16, isOutput=False)
    id_d = nc.declare_dram_parameter("identf", [128, 128], F32, isOutput=False)
    outT_d = nc.declare_dram_parameter("outT", [D_MODEL, T], F16, isOutput=True)

    with tile.TileContext(nc) as tc, ExitStack() as ctx:
        consts = ctx.enter_context(tc.tile_pool(name="consts", bufs=1))
        big = ctx.enter_context(tc.tile_pool(name="big", bufs=1))
        ptp = ctx.enter_context(tc.tile_pool(name="ptp", bufs=4))
        rp = ctx.enter_context(tc.tile_pool(name="rp", bufs=3))
        osb = ctx.enter_context(tc.tile_pool(name="osb", bufs=4))
        sp = ctx.enter_context(tc.tile_pool(name="sp", bufs=2, space="PSUM"))
        avp = ctx.enter_context(tc.tile_pool(name="avp", bufs=1, space="PSUM"))
        dramp = ctx.enter_context(tc.tile_pool(name="dramp", bufs=2, space="DRAM"))

        xT_sb = big.tile([128, NQT * BLK], BF16, tag="xTb")
        w_sb = consts.tile([128, 3 * CCH * 128], BF16, tag="w")
        b_sb = consts.tile([128, 3], F32, tag="b")
        bvr_sb = consts.tile([1, 128], BF16, tag="bvr")
        wo_sb = consts.tile([128, D_MODEL], BF16, tag="wo")
        mask_sb = consts.tile([128, 4 * QT], BF16, tag="mask")
        ones_sb = consts.tile([1, QT], BF16, tag="ones1")
        id_sb = consts.tile([128, 128], F32, tag="identf")

        # w + b + first x block first so projections start ASAP
        nc.sync.dma_start(out=w_sb[:], in_=w_d[:, :])
        nc.sync.dma_start(out=b_sb[:], in_=b_d[:, :])
        nc.sync.dma_start(out=xT_sb[:, 0:BLK], in_=xT_d[:, 0:BLK])
        nc.sync.dma_start(out=wo_sb[:], in_=wo_d[:, :])
        nc.sync.dma_start(out=mask_sb[:], in_=mk_d[:, :])
        nc.sync.dma_start(out=bvr_sb[:], in_=bvr_d[:, :])
        nc.sync.dma_start(out=id_sb[:], in_=id_d[:, :])
        nc.vector.memset(ones_sb[:], 1.0)

        # warm the PE (HAM clock gate) during the input-DMA window with
        # throwaway rank-1 matmuls; results land in a psum tile nobody reads
        warm = avp.tile([128, 2 * QT], F32, tag="av")
        for _ in range(28):
            nc.tensor.matmul(
                warm[0:128, 0:QT], ones_sb[:, 0:128], ones_sb[:, 0:QT],
                start=True, stop=True,
            )

        QT_sb = big.tile([128, T], BF16, tag="qsb")
        KT_sb = big.tile([128, T], BF16, tag="ksb")
        V2 = big.tile([128, (T // KC) * VST], BF16, tag="V2")
        nc.vector.memset(V2[:], 0.0)
        v3 = V2[:].rearrange("p (t c) -> p t c", c=VST)
        nc.vector.memset(v3[:, :, 64:65], 1.0)
        nc.vector.memset(v3[:, :, 97:98], 1.0)
        ho_u = big.tile([128, T], F32, tag="ho_u")      # unnormalized AV
        hob = big.tile([128, T], BF16, tag="hob")       # normalized, bf16
        dn_st = big.tile([128, 2 * QT], F32, tag="dn")  # denom staging
        nc.vector.memset(dn_st[:], 0.0)

        rounds = {}  # round key -> state

        def xchunk(tt, j, lo, hi):
            base = tt * BLK + j * QT
            return xT_sb[:, base + lo:base + hi]

        def stage_a(qts):
            # denominator rows (dA row 64 bank0, dB row 32 bank1, already
            # staged in dn_st) -> DRAM rd -> partition-spread dn_sb so one
            # [128, 16] reciprocal covers up to 2 qtiles x 2 heads
            n = len(qts)
            c0 = (qts[0] % 2) * QT
            rd = dramp.tile([2, 2 * QT], F32, tag="rd")
            nc.sync.dma_start(out=rd[0:1, 0:n * QT],
                              in_=dn_st[64:65, c0:c0 + n * QT])
            nc.sync.dma_start(out=rd[1:2, 0:n * QT],
                              in_=dn_st[32:33, c0:c0 + n * QT])
            dn_sb = rp.tile([128, 16], F32, tag="dnsb")
            for r in range(2):  # rd row r at linear offset r*2*QT
                nc.sync.dma_start(
                    out=dn_sb[:, r * 8:r * 8 + 4 * n],
                    in_=bass.AP(tensor=rd.tensor, offset=rd.offset + r * 2 * QT,
                                ap=[[4 * n, 128], [1, 4 * n]]))
            rounds[tuple(qts)] = {"dn_sb": dn_sb}

        def stage_b(qts):
            n = len(qts)
            st = rounds[tuple(qts)]
            dn_r = rp.tile([128, 16], F32, tag="dnr")
            nc.vector.reciprocal(dn_r[:, 0:16], st["dn_sb"][:, 0:16])
            rr = dramp.tile([1, 4 * QT], F32, tag="rr")
            for r in range(2):
                nc.sync.dma_start(
                    out=bass.AP(tensor=rr.tensor, offset=rr.offset + r * n * QT,
                                ap=[[4 * n, 128], [1, 4 * n]]),
                    in_=dn_r[:, r * 8:r * 8 + 4 * n])
            # rr linear: [rA(qts[0]) .. rA(qts[n-1]) | rB(qts[0]) .. ]
            rbcs = []
            for i in range(n):
                rbc = rp.tile([128, QT], F32, tag="rbc")
                rA = rr[0:1, i * QT:(i + 1) * QT]
                rB = rr[0:1, (n + i) * QT:(n + i + 1) * QT]
                nc.gpsimd.dma_start(
                    out=rbc[0:64, :],
                    in_=bass.AP(tensor=rA.tensor, offset=rA.offset,
                                ap=[[0, 64]] + list(rA.ap[1:])))
                nc.gpsimd.dma_start(
                    out=rbc[64:128, :],
                    in_=bass.AP(tensor=rB.tensor, offset=rB.offset,
                                ap=[[0, 64]] + list(rB.ap[1:])))
                rbcs.append(rbc)
            st["rbcs"] = rbcs

        def stage_c(qts):
            st = rounds.pop(tuple(qts))
            for i, q2 in enumerate(qts):
                q2s = q2 * QT
                nc.vector.tensor_mul(
                    hob[:, q2s:q2s + QT], ho_u[:, q2s:q2s + QT],
                    st["rbcs"][i][:])
                for dch in range(CCH):
                    op = sp.tile([128, QT], F32, tag="sc")
                    nc.tensor.matmul(
                        op[:], wo_sb[:, dch * 128:(dch + 1) * 128],
                        hob[:, q2s:q2s + QT], start=True, stop=True,
                    )
                    ot = osb.tile([128, QT], F16, tag="ot")
                    nc.vector.tensor_copy(ot[:], op[:])
                    nc.sync.dma_start(
                        out=outT_d[dch * 128:(dch + 1) * 128, q2s:q2s + QT],
                        in_=ot[:],
                    )

        def proj_unit(tt, s):
            dst = QT_sb if s == 0 else KT_sb
            with nc.named_scope("proj"):
                pp = sp.tile([128, QT], F32, tag="sc")
                for j in range(CCH):
                    nc.tensor.matmul(
                        pp[:],
                        w_sb[:, (s * CCH + j) * 128:(s * CCH + j + 1) * 128],
                        xchunk(tt, j, 0, QT),
                        start=(j == 0), stop=(j == CCH - 1),
                    )
                nc.vector.tensor_scalar_add(
                    dst[:, tt * QT:(tt + 1) * QT], pp[:], b_sb[:, s:s + 1])

        def vt_unit(tt, q4):
            with nc.named_scope("vt"):
                tt4 = tt * 4 + q4
                vt = sp.tile([128, 128], F32, tag="sc")
                for j in range(CCH):
                    base = (2 * CCH + j) * 128
                    nc.tensor.matmul(
                        vt[:], xchunk(tt, j, q4 * KC, (q4 + 1) * KC),
                        w_sb[:, base:base + 128],
                        start=(j == 0), stop=False,
                    )
                nc.tensor.matmul(  # rank-1 bias: out[tok, :] += bv
                    vt[:], ones_sb[:, 0:128], bvr_sb[:, :],
                    start=False, stop=True,
                )
                nc.vector.tensor_copy(
                    V2[:, tt4 * VST:tt4 * VST + 64], vt[:, 0:64])
                nc.vector.tensor_copy(
                    V2[:, tt4 * VST + 129:tt4 * VST + 193], vt[:, 64:128])

        # ================= main token-block pipeline =================
        # block 0 work runs up front; block tt+1 proj/V^T are emitted as
        # fillers between attention groups of qtile tt (the early qtiles are
        # ACT-limited, so the PE absorbs them for free)
        for s in range(2):
            proj_unit(0, s)
        for q4 in range(4):
            vt_unit(0, q4)
        for tt in range(NQT):
            if tt + 1 < NQT:
                nc.sync.dma_start(
                    out=xT_sb[:, (tt + 1) * BLK:(tt + 2) * BLK],
                    in_=xT_d[:, (tt + 1) * BLK:(tt + 2) * BLK])

            # ---- attention for query tile qi = tt ----
            qi = tt
            qs = qi * QT
            # av bank 0: head-A group (AV rows 0:64, denom row 64)
            # av bank 1: head-B group (denom row 32, AV rows 64:128)
            av = avp.tile([128, 2 * QT], F32, tag="av")
            nsteps = 4 * (qi + 1)
            jobs = [(kc, h) for kc in range(nsteps) for h in (0, 1)]
            # stage work injected between qtile-7 attention groups
            mid = {}
            if qi == 7:
                mid = {4: lambda: stage_b((4, 5)), 8: lambda: stage_b((6,)),
                       12: lambda: stage_c((4, 5)), 16: lambda: stage_c((6,))}
            for gn, g in enumerate(range(0, len(jobs), GRP)):
                if gn in mid:
                    mid[gn]()
                grp = jobs[g:g + GRP]
                width = len(grp) * QT
                with nc.named_scope("score"):
                    sc = sp.tile([128, GRP * QT], F32, tag="sc")
                    for ji, (kc, h) in enumerate(grp):
                        nc.tensor.matmul(
                            sc[:, ji * QT:(ji + 1) * QT],
                            KT_sb[64 * h:64 * h + 64, kc * KC:(kc + 1) * KC],
                            QT_sb[64 * h:64 * h + 64, qs:qs + QT],
                            start=True, stop=True, tile_position=(64 * h, 0),
                        )
                pt = ptp.tile([128, GRP * QT], BF16, tag="pt")
                with nc.named_scope("exp"):
                    nc.scalar.activation(
                        pt[:, :width], sc[:, :width], AF.Exp,
                        scale=1.0 / math.sqrt(HEAD_DIM),
                    )
                with nc.named_scope("av"):
                    for ji, (kc, h) in enumerate(grp):
                        if kc >= 4 * qi:  # diagonal straddle
                            ptj = pt[:, ji * QT:(ji + 1) * QT]
                            pat = kc - 4 * qi
                            m = mask_sb[:, pat * QT:(pat + 1) * QT]
                            nc.vector.tensor_mul(ptj, ptj, m)
                    for ji, (kc, h) in enumerate(grp):
                        ptj = pt[:, ji * QT:(ji + 1) * QT]
                        st_ = kc == 0
                        sp_ = kc == nsteps - 1
                        vbase = kc * VST
                        if h == 0:
                            # lhsT padded to 128 cols; rows 65:128 junk
                            nc.tensor.matmul(
                                av[0:128, 0:QT], V2[:, vbase:vbase + 128], ptj,
                                start=st_, stop=sp_, tile_position=(0, 0),
                            )
                        else:
                            nc.tensor.matmul(
                                av[0:128, QT:2 * QT],
                                V2[:, vbase + 65:vbase + 193],
                                ptj, start=st_, stop=sp_, tile_position=(0, 0),
                            )
            if tt + 1 < NQT:
                for s in range(2):
                    proj_unit(tt + 1, s)
                for q4 in range(4):
                    vt_unit(tt + 1, q4)
            # stash unnormalized AV + denominators, all on DVE: the ACT
            # queue must stay pure exp (attention is a tight PE<->ACT race;
            # 1.4us of copies there stalls the PE via the score ring).
            # For the last qtile the denom copies go first: the final
            # normalization chain starts from them.
            with nc.named_scope("stash"):
                dcol = (qi % 2) * QT
                copies = [
                    lambda: nc.vector.tensor_copy(
                        dn_st[64:65, dcol:dcol + QT], av[64:65, 0:QT]),
                    lambda: nc.vector.tensor_copy(
                        dn_st[32:33, dcol:dcol + QT], av[32:33, QT:2 * QT]),
                    lambda: nc.vector.tensor_copy(
                        ho_u[0:64, qs:qs + QT], av[0:64, 0:QT]),
                    lambda: nc.vector.tensor_copy(
                        ho_u[64:128, qs:qs + QT], av[64:128, QT:2 * QT]),
                ]
                if qi != 7:
                    copies = copies[2:] + copies[:2]
                for c in copies:
                    c()

            # staged deferred normalization + out-projection
            with nc.named_scope("norm"):
                if qi == 1:
                    stage_a((0, 1))
                elif qi == 2:
                    stage_b((0, 1))
                elif qi == 3:
                    stage_c((0, 1))
                    stage_a((2, 3))
                elif qi == 4:
                    stage_b((2, 3))
                elif qi == 5:
                    stage_c((2, 3))
                    stage_a((4, 5))
                elif qi == 6:
                    stage_a((6,))
                elif qi == 7:
                    # on-chip final round: PE transposes spread the
                    # denominators across partitions, one [128, 8]
                    # reciprocal, column transposes bring the recips back
                    # to partition 0, rank-1 matmuls broadcast them; no
                    # DRAM bounce in the serial tail
                    tp = sp.tile([128, GRP * QT], F32, tag="sc")
                    for k in range(4):
                        nc.tensor.transpose(
                            tp[:, k * 128:(k + 1) * 128],
                            dn_st[:, QT + k * 128:QT + (k + 1) * 128],
                            id_sb[:])
                    r8 = rp.tile([128, 16], F32, tag="dnsb")
                    selA = tp[:, 64:65]
                    selB = tp[:, 32:33]
                    nc.vector.reciprocal(
                        r8[:, 0:4],
                        bass.AP(tensor=selA.tensor, offset=selA.offset,
                                ap=[list(selA.ap[0]), [128, 4]]))
                    nc.vector.reciprocal(
                        r8[:, 4:8],
                        bass.AP(tensor=selB.tensor, offset=selB.offset,
                                ap=[list(selB.ap[0]), [128, 4]]))
                    for k in range(8):
                        nc.tensor.transpose(
                            tp[0:1, QT + k * 128:QT + (k + 1) * 128],
                            r8[:, k:k + 1], id_sb[:])
                    rr_row = rp.tile([1, 2 * QT], BF16, tag="rrow")
                    nc.vector.tensor_copy(
                        rr_row[0:1, :], tp[0:1, QT:QT + 2 * QT])
                    rbc_ps = sp.tile([128, 2 * QT], F32, tag="sc")
                    for k in range(4):
                        nc.tensor.matmul(
                            rbc_ps[0:64, k * 128:(k + 1) * 128],
                            ones_sb[0:1, 0:64],
                            rr_row[0:1, k * 128:(k + 1) * 128],
                            start=True, stop=True, tile_position=(0, 0),
                        )
                        nc.tensor.matmul(
                            rbc_ps[64:128, QT + k * 128:QT + (k + 1) * 128],
                            ones_sb[0:1, 0:64],
                            rr_row[0:1, (4 + k) * 128:(5 + k) * 128],
                            start=True, stop=True, tile_position=(0, 64),
                        )
                    nc.vector.tensor_mul(
                        hob[0:64, qs:qs + QT], ho_u[0:64, qs:qs + QT],
                        rbc_ps[0:64, 0:QT])
                    nc.vector.tensor_mul(
                        hob[64:128, qs:qs + QT], ho_u[64:128, qs:qs + QT],
                        rbc_ps[64:128, QT:2 * QT])
                    for dch in range(CCH):
                        op = sp.tile([128, QT], F32, tag="sc")
                        nc.tensor.matmul(
                            op[:], wo_sb[:, dch * 128:(dch + 1) * 128],
                            hob[:, qs:qs + QT], start=True, stop=True,
                        )
                        ot = osb.tile([128, QT], F16, tag="ot")
                        nc.vector.tensor_copy(ot[:], op[:])
                        nc.sync.dma_start(
                            out=outT_d[dch * 128:(dch + 1) * 128,
                                       qs:qs + QT],
                            in_=ot[:],
                        )
    nc.finalize()
    return nc


def _host_inputs(x, wq, bq, wk, bk, wv, bv, wo):
    """Per-core input maps. Slot A of core c = head c; slot B = head 8+c
    (cores 0-3) or a dummy zero head (cores 4-7)."""
    bf16 = ml_dtypes.bfloat16
    # block-major xTb: [128, tt*3072 + j*512 + i] = x[tt*512+i, j*128+p]
    xt = x[0].reshape(NQT, QT, CCH, 128)          # [tt, i, j, p]
    xTb = np.ascontiguousarray(
        xt.transpose(3, 0, 2, 1).reshape(128, NQT * BLK)).astype(bf16)
    masks = np.zeros((4, 128, QT), np.float32)
    dk = np.arange(128)[:, None]
    dq = np.arange(QT)[None, :]
    for p in range(4):
        masks[p] = (dk + 128 * p <= dq)
    masks2 = np.ascontiguousarray(
        masks.transpose(1, 0, 2).reshape(128, 4 * QT)).astype(bf16)

    in_maps = []
    for c in range(N_CORES):
        hA = c
        hB = 8 + c if c < 4 else None
        w = np.zeros((3, D_MODEL, 128), np.float32)
        b = np.zeros((128, 3), np.float32)
        bvrow = np.zeros((1, 128), np.float32)
        wo2 = np.zeros((128, D_MODEL), np.float32)
        for s, (W, B) in enumerate(((wq, bq), (wk, bk), (wv, bv))):
            w[s, :, 0:64] = W[hA]
            if s < 2:
                b[0:64, s] = B[hA]
            if hB is not None:
                w[s, :, 64:128] = W[hB]
                if s < 2:
                    b[64:128, s] = B[hB]
        bvrow[0, 0:64] = bv[hA]
        if hB is not None:
            bvrow[0, 64:128] = bv[hB]
        wo2[0:64, :] = wo[hA * 64:(hA + 1) * 64, :]
        if hB is not None:
            wo2[64:128, :] = wo[hB * 64:(hB + 1) * 64, :]
        # w2[p, (s*6+j)*128 + d] = w[s, j*128+p, d]
        w2 = np.ascontiguousarray(
            w.reshape(3, CCH, 128, 128).transpose(2, 0, 1, 3)
            .reshape(128, 3 * CCH * 128)).astype(bf16)
        in_maps.append({
            "xTb": xTb,
            "w2": w2,
            "bqkv": b.astype(np.float32),
            "bvrow": bvrow.astype(bf16),
            "wo2": wo2.astype(bf16),
            "masks2": masks2,
            "identf": np.eye(128, dtype=np.float32),
        })
    return in_maps


def kernel(_trace=False, _tmpdir=None, **inputs):
    x = np.asarray(inputs["x"], np.float32)
    args = (x,
            np.asarray(inputs["wq"], np.float32), np.asarray(inputs["bq"], np.float32),
            np.asarray(inputs["wk"], np.float32), np.asarray(inputs["bk"], np.float32),
            np.asarray(inputs["wv"], np.float32), np.asarray(inputs["bv"], np.float32),
            np.asarray(inputs["wo"], np.float32))
    bo = np.asarray(inputs["bo"], np.float32)

    if "nc" not in _PROGRAM_CACHE:
        _PROGRAM_CACHE["nc"] = build_program()
    nc = _PROGRAM_CACHE["nc"]

    in_maps = _host_inputs(*args)
    res = run_bass_kernel_spmd(
        nc, in_maps, list(range(N_CORES)), trace=_trace, tmpdir=_tmpdir,
    )
    acc = np.zeros((D_MODEL, T), np.float32)
    for c in range(N_CORES):
        acc += res.results[c]["outT"].astype(np.float32)
    out = acc.T + bo[None, :]
    if _trace:
        return out[None].astype(np.float32), res
    return out[None].astype(np.float32)


# revision 20
# speedup vs baseline: 1.0248x; 1.0248x over previous
"""Multi-head causal attention (B=1, T=4096, D=768, H=12) on 8 trn2 cores.

Sharding: 16 uniform head-slots (2 per core), 12 real heads + 4 dummy
(zero-weight) slots.  Every core runs the IDENTICAL program (SPMD); cores
differ only in the weight data they receive.  Each core computes, for its
two head-slots, the full causal attention over all 4096 tokens plus that
slot-pair's partial output projection (out.T = wo_slice.T @ headout).  The
host sums the 8 partial [768, 4096] fp16 outputs, transposes, adds bias.

Pipeline structure (v4): x arrives in 8 token blocks (block-major DRAM
layout, one 128-line DMA per block; all constants are host-packed so each
needs a single DMA).  Per block tt: project Q/K for those 512 tokens,
build V^T directly via transposed matmuls (lhsT = x chunk), then run
attention for query tile tt.  Softmax normalization is deferred and staged
across later emission points (denominator bounce -> batched reciprocal ->
partition broadcast -> scale + out-projection); the last two query tiles
get their own single-tile rounds, partially emitted between qtile-7
attention groups, to shrink the serial tail.
"""

import math
import numpy as np
import ml_dtypes
from contextlib import ExitStack

import concourse.bass as bass
import concourse.bacc as bacc
import concourse.mybir as mybir
import concourse.tile as tile
from concourse.bass_utils import run_bass_kernel_spmd

BF16 = mybir.dt.bfloat16
F16 = mybir.dt.float16
F32 = mybir.dt.float32
AF = mybir.ActivationFunctionType

T = 4096
D_MODEL = 768
HEAD_DIM = 64
N_HEADS = 12
N_CORES = 8
QT = 512                  # query tile width (one full PSUM bank per chunk)
KC = 128                  # key chunk (psum partition dim)
GRP = 3                   # score chunk-jobs per exp group -> ACT free dim 1536
NQT = T // QT             # 8 query tiles / token blocks
CCH = D_MODEL // 128      # 6 contraction chunks
BLK = QT * CCH            # 3072 cols per block in xTb layout
VST = 208                 # V2 column stride per key chunk

_PROGRAM_CACHE = {}


def build_program():
    nc = bacc.Bacc(None)

    # xTb block-major: xTb[p, tt*3072 + j*512 + i] = x[tt*512 + i, j*128 + p]
    xT_d = nc.declare_dram_parameter("xTb", [128, NQT * BLK], BF16, isOutput=False)
    # w2 pre-packed in SBUF layout: w2[p, (s*6+j)*128 + d] = W_s[j*128+p, d]
    w_d = nc.declare_dram_parameter("w2", [128, 3 * CCH * 128], BF16, isOutput=False)
    b_d = nc.declare_dram_parameter("bqkv", [128, 3], F32, isOutput=False)
    bvr_d = nc.declare_dram_parameter("bvrow", [1, 128], BF16, isOutput=False)
    wo_d = nc.declare_dram_parameter("wo2", [128, D_MODEL], BF16, isOutput=False)
    mk_d = nc.declare_dram_parameter("masks2", [128, 4 * QT], BF16, isOutput=False)
    id_d = nc.declare_dram_parameter("identf", [128, 128], F32, isOutput=False)
    outT_d = nc.declare_dram_parameter("outT", [D_MODEL, T], F16, isOutput=True)

    with tile.TileContext(nc) as tc, ExitStack() as ctx:
        consts = ctx.enter_context(tc.tile_pool(name="consts", bufs=1))
        big = ctx.enter_context(tc.tile_pool(name="big", bufs=1))
        ptp = ctx.enter_context(tc.tile_pool(name="ptp", bufs=4))
        rp = ctx.enter_context(tc.tile_pool(name="rp", bufs=3))
        osb = ctx.enter_context(tc.tile_pool(name="osb", bufs=4))
        sp = ctx.enter_context(tc.tile_pool(name="sp", bufs=2, space="PSUM"))
        avp = ctx.enter_context(tc.tile_pool(name="avp", bufs=1, space="PSUM"))
        dramp = ctx.enter_context(tc.tile_pool(name="dramp", bufs=2, space="DRAM"))

        xT_sb = big.tile([128, NQT * BLK], BF16, tag="xTb")
        w_sb = consts.tile([128, 3 * CCH * 128], BF16, tag="w")
        b_sb = consts.tile([128, 3], F32, tag="b")
        bvr_sb = consts.tile([1, 128], BF16, tag="bvr")
        wo_sb = consts.tile([128, D_MODEL], BF16, tag="wo")
        mask_sb = consts.tile([128, 4 * QT], BF16, tag="mask")
        ones_sb = consts.tile([1, QT], BF16, tag="ones1")
        id_sb = consts.tile([128, 128], F32, tag="identf")

        # w + b + first x block first so projections start ASAP
        nc.sync.dma_start(out=w_sb[:], in_=w_d[:, :])
        nc.sync.dma_start(out=b_sb[:], in_=b_d[:, :])
        nc.sync.dma_start(out=xT_sb[:, 0:BLK], in_=xT_d[:, 0:BLK])
        nc.sync.dma_start(out=wo_sb[:], in_=wo_d[:, :])
        nc.sync.dma_start(out=mask_sb[:], in_=mk_d[:, :])
        nc.sync.dma_start(out=bvr_sb[:], in_=bvr_d[:, :])
        nc.sync.dma_start(out=id_sb[:], in_=id_d[:, :])
        nc.vector.memset(ones_sb[:], 1.0)

        # warm the PE (HAM clock gate) during the input-DMA window with
        # throwaway rank-1 matmuls; results land in a psum tile nobody reads
        warm = avp.tile([128, 2 * QT], F32, tag="av")
        for _ in range(28):
            nc.tensor.matmul(
                warm[0:128, 0:QT], ones_sb[:, 0:128], ones_sb[:, 0:QT],
                start=True, stop=True,
            )

        QT_sb = big.tile([128, T], BF16, tag="qsb")
        KT_sb = big.tile([128, T], BF16, tag="ksb")
        V2 = big.tile([128, (T // KC) * VST], BF16, tag="V2")
        nc.vector.memset(V2[:], 0.0)
        v3 = V2[:].rearrange("p (t c) -> p t c", c=VST)
        nc.vector.memset(v3[:, :, 64:65], 1.0)
        nc.vector.memset(v3[:, :, 97:98], 1.0)
        ho_u = big.tile([128, T], F32, tag="ho_u")      # unnormalized AV
        hob = big.tile([128, T], BF16, tag="hob")       # normalized, bf16
        dn_st = big.tile([128, 2 * QT], F32, tag="dn")  # denom staging
        nc.vector.memset(dn_st[:], 0.0)

        rounds = {}  # round key -> state

        def xchunk(tt, j, lo, hi):
            base = tt * BLK + j * QT
            return xT_sb[:, base + lo:base + hi]

        def stage_a(qts):
            # denominator rows (dA row 64 bank0, dB row 32 bank1, already
            # staged in dn_st) -> DRAM rd -> partition-spread dn_sb so one
            # [128, 16] reciprocal covers up to 2 qtiles x 2 heads
            n = len(qts)
            c0 = (qts[0] % 2) * QT
            rd = dramp.tile([2, 2 * QT], F32, tag="rd")
            nc.sync.dma_start(out=rd[0:1, 0:n * QT],
                              in_=dn_st[64:65, c0:c0 + n * QT])
            nc.sync.dma_start(out=rd[1:2, 0:n * QT],
                              in_=dn_st[32:33, c0:c0 + n * QT])
            dn_sb = rp.tile([128, 16], F32, tag="dnsb")
            for r in range(2):  # rd row r at linear offset r*2*QT
                nc.sync.dma_start(
                    out=dn_sb[:, r * 8:r * 8 + 4 * n],
                    in_=bass.AP(tensor=rd.tensor, offset=rd.offset + r * 2 * QT,
                                ap=[[4 * n, 128], [1, 4 * n]]))
            rounds[tuple(qts)] = {"dn_sb": dn_sb}

        def stage_b(qts):
            n = len(qts)
            st = rounds[tuple(qts)]
            dn_r = rp.tile([128, 16], F32, tag="dnr")
            nc.vector.reciprocal(dn_r[:, 0:16], st["dn_sb"][:, 0:16])
            rr = dramp.tile([1, 4 * QT], F32, tag="rr")
            for r in range(2):
                nc.sync.dma_start(
                    out=bass.AP(tensor=rr.tensor, offset=rr.offset + r * n * QT,
                                ap=[[4 * n, 128], [1, 4 * n]]),
                    in_=dn_r[:, r * 8:r * 8 + 4 * n])
            # rr linear: [rA(qts[0]) .. rA(qts[n-1]) | rB(qts[0]) .. ]
            rbcs = []
            for i in range(n):
                rbc = rp.tile([128, QT], F32, tag="rbc")
                rA = rr[0:1, i * QT:(i + 1) * QT]
                rB = rr[0:1, (n + i) * QT:(n + i + 1) * QT]
                nc.gpsimd.dma_start(
                    out=rbc[0:64, :],
                    in_=bass.AP(tensor=rA.tensor, offset=rA.offset,
                                ap=[[0, 64]] + list(rA.ap[1:])))
                nc.gpsimd.dma_start(
                    out=rbc[64:128, :],
                    in_=bass.AP(tensor=rB.tensor, offset=rB.offset,
                                ap=[[0, 64]] + list(rB.ap[1:])))
                rbcs.append(rbc)
            st["rbcs"] = rbcs

        def stage_c(qts):
            st = rounds.pop(tuple(qts))
            for i, q2 in enumerate(qts):
                q2s = q2 * QT
                nc.vector.tensor_mul(
                    hob[:, q2s:q2s + QT], ho_u[:, q2s:q2s + QT],
                    st["rbcs"][i][:])
                for dch in range(CCH):
                    op = sp.tile([128, QT], F32, tag="sc")
                    nc.tensor.matmul(
                        op[:], wo_sb[:, dch * 128:(dch + 1) * 128],
                        hob[:, q2s:q2s + QT], start=True, stop=True,
                    )
                    ot = osb.tile([128, QT], F16, tag="ot")
                    nc.vector.tensor_copy(ot[:], op[:])
                    nc.sync.dma_start(
                        out=outT_d[dch * 128:(dch + 1) * 128, q2s:q2s + QT],
                        in_=ot[:],
                    )

        def proj_unit(tt, s):
            dst = QT_sb if s == 0 else KT_sb
            with nc.named_scope("proj"):
                pp = sp.tile([128, QT], F32, tag="sc")
                for j in range(CCH):
                    nc.tensor.matmul(
                        pp[:],
                        w_sb[:, (s * CCH + j) * 128:(s * CCH + j + 1) * 128],
                        xchunk(tt, j, 0, QT),
                        start=(j == 0), stop=(j == CCH - 1),
                    )
                nc.vector.tensor_scalar_add(
                    dst[:, tt * QT:(tt + 1) * QT], pp[:], b_sb[:, s:s + 1])

        def vt_unit(tt, q4):
            with nc.named_scope("vt"):
                tt4 = tt * 4 + q4
                vt = sp.tile([128, 128], F32, tag="sc")
                for j in range(CCH):
                    base = (2 * CCH + j) * 128
                    nc.tensor.matmul(
                        vt[:], xchunk(tt, j, q4 * KC, (q4 + 1) * KC),
                        w_sb[:, base:base + 128],
                        start=(j == 0), stop=False,
                    )
                nc.tensor.matmul(  # rank-1 bias: out[tok, :] += bv
                    vt[:], ones_sb[:, 0:128], bvr_sb[:, :],
                    start=False, stop=True,
                )
                nc.vector.tensor_copy(
                    V2[:, tt4 * VST:tt4 * VST + 64], vt[:, 0:64])
                nc.vector.tensor_copy(
                    V2[:, tt4 * VST + 129:tt4 * VST + 193], vt[:, 64:128])

        # ================= main token-block pipeline =================
        # block 0 work runs up front; block tt+1 proj/V^T are emitted as
        # fillers between attention groups of qtile tt (the early qtiles are
        # ACT-limited, so the PE absorbs them for free)
        for s in range(2):
            proj_unit(0, s)
        for q4 in range(4):
            vt_unit(0, q4)
        for tt in range(NQT):
            if tt + 1 < NQT:
                nc.sync.dma_start(
                    out=xT_sb[:, (tt + 1) * BLK:(tt + 2) * BLK],
                    in_=xT_d[:, (tt + 1) * BLK:(tt + 2) * BLK])

            # ---- attention for query tile qi = tt ----
            qi = tt
            qs = qi * QT
            # av bank 0: head-A group (AV rows 0:64, denom row 64)
            # av bank 1: head-B group (denom row 32, AV rows 64:128)
            av = avp.tile([128, 2 * QT], F32, tag="av")
            nsteps = 4 * (qi + 1)
            jobs = [(kc, h) for kc in range(nsteps) for h in (0, 1)]
            # stage work injected between qtile-7 attention groups
            mid = {}
            if qi == 7:
                mid = {4: lambda: stage_b((4, 5)), 8: lambda: stage_b((6,)),
                       12: lambda: stage_c((4, 5)), 16: lambda: stage_c((6,))}
            def emit_scores(grp):
                width = len(grp) * QT
                with nc.named_scope("score"):
                    sc = sp.tile([128, GRP * QT], F32, tag="sc")
                    for ji, (kc, h) in enumerate(grp):
                        nc.tensor.matmul(
                            sc[:, ji * QT:(ji + 1) * QT],
                            KT_sb[64 * h:64 * h + 64, kc * KC:(kc + 1) * KC],
                            QT_sb[64 * h:64 * h + 64, qs:qs + QT],
                            start=True, stop=True, tile_position=(64 * h, 0),
                        )
                pt = ptp.tile([128, GRP * QT], BF16, tag="pt")
                with nc.named_scope("exp"):
                    nc.scalar.activation(
                        pt[:, :width], sc[:, :width], AF.Exp,
                        scale=1.0 / math.sqrt(HEAD_DIM),
                    )
                with nc.named_scope("av"):
                    for ji, (kc, h) in enumerate(grp):
                        if kc >= 4 * qi:  # diagonal straddle
                            ptj = pt[:, ji * QT:(ji + 1) * QT]
                            pat = kc - 4 * qi
                            m = mask_sb[:, pat * QT:(pat + 1) * QT]
                            nc.vector.tensor_mul(ptj, ptj, m)
                return grp, pt

            def emit_av(grp, pt):
                with nc.named_scope("av"):
                    for ji, (kc, h) in enumerate(grp):
                        ptj = pt[:, ji * QT:(ji + 1) * QT]
                        st_ = kc == 0
                        sp_ = kc == nsteps - 1
                        vbase = kc * VST
                        if h == 0:
                            # lhsT padded to 128 cols; rows 65:128 junk
                            nc.tensor.matmul(
                                av[0:128, 0:QT], V2[:, vbase:vbase + 128], ptj,
                                start=st_, stop=sp_, tile_position=(0, 0),
                            )
                        else:
                            nc.tensor.matmul(
                                av[0:128, QT:2 * QT],
                                V2[:, vbase + 65:vbase + 193],
                                ptj, start=st_, stop=sp_, tile_position=(0, 0),
                            )

            # score groups emitted in adjacent pairs (6 back-to-back score
            # matmuls -> every 64-row half pairs with a concurrent partner),
            # with each pair's AV matmuls deferred until after the NEXT
            # pair's scores, so AV never waits on the exp of its own group
            pending = []
            for gn, g in enumerate(range(0, len(jobs), GRP)):
                if gn in mid:
                    mid[gn]()
                if gn % 2 == 0 and len(pending) == 2:
                    for item in pending:
                        emit_av(*item)
                    pending = []
                pending.append(emit_scores(jobs[g:g + GRP]))
            for item in pending:
                emit_av(*item)
            if tt + 1 < NQT:
                for s in range(2):
                    proj_unit(tt + 1, s)
                for q4 in range(4):
                    vt_unit(tt + 1, q4)
            # stash unnormalized AV (DVE) + denominators (ScalarE, parallel)
            with nc.named_scope("stash"):
                dcol = (qi % 2) * QT
                nc.scalar.copy(dn_st[64:65, dcol:dcol + QT], av[64:65, 0:QT])
                nc.scalar.copy(dn_st[32:33, dcol:dcol + QT],
                               av[32:33, QT:2 * QT])
                nc.vector.tensor_copy(ho_u[0:64, qs:qs + QT], av[0:64, 0:QT])
                nc.vector.tensor_copy(
                    ho_u[64:128, qs:qs + QT], av[64:128, QT:2 * QT])

            # staged deferred normalization + out-projection
            with nc.named_scope("norm"):
                if qi == 1:
                    stage_a((0, 1))
                elif qi == 2:
                    stage_b((0, 1))
                elif qi == 3:
                    stage_c((0, 1))
                    stage_a((2, 3))
                elif qi == 4:
                    stage_b((2, 3))
                elif qi == 5:
                    stage_c((2, 3))
                    stage_a((4, 5))
                elif qi == 6:
                    stage_a((6,))
                elif qi == 7:
                    # on-chip final round: PE transposes spread the
                    # denominators across partitions, one [128, 8]
                    # reciprocal, column transposes bring the recips back
                    # to partition 0, rank-1 matmuls broadcast them; no
                    # DRAM bounce in the serial tail
                    tp = sp.tile([128, GRP * QT], F32, tag="sc")
                    for k in range(4):
                        nc.tensor.transpose(
                            tp[:, k * 128:(k + 1) * 128],
                            dn_st[:, QT + k * 128:QT + (k + 1) * 128],
                            id_sb[:])
                    r8 = rp.tile([128, 16], F32, tag="dnsb")
                    selA = tp[:, 64:65]
                    selB = tp[:, 32:33]
                    nc.vector.reciprocal(
                        r8[:, 0:4],
                        bass.AP(tensor=selA.tensor, offset=selA.offset,
                                ap=[list(selA.ap[0]), [128, 4]]))
                    nc.vector.reciprocal(
                        r8[:, 4:8],
                        bass.AP(tensor=selB.tensor, offset=selB.offset,
                                ap=[list(selB.ap[0]), [128, 4]]))
                    for k in range(8):
                        nc.tensor.transpose(
                            tp[0:1, QT + k * 128:QT + (k + 1) * 128],
                            r8[:, k:k + 1], id_sb[:])
                    rr_row = rp.tile([1, 2 * QT], BF16, tag="rrow")
                    nc.vector.tensor_copy(
                        rr_row[0:1, :], tp[0:1, QT:QT + 2 * QT])
                    rbc_ps = sp.tile([128, 2 * QT], F32, tag="sc")
                    for k in range(4):
                        nc.tensor.matmul(
                            rbc_ps[0:64, k * 128:(k + 1) * 128],
                            ones_sb[0:1, 0:64],
                            rr_row[0:1, k * 128:(k + 1) * 128],
                            start=True, stop=True, tile_position=(0, 0),
                        )
                        nc.tensor.matmul(
                            rbc_ps[64:128, QT + k * 128:QT + (k + 1) * 128],
                            ones_sb[0:1, 0:64],
                            rr_row[0:1, (4 + k) * 128:(5 + k) * 128],
                            start=True, stop=True, tile_position=(0, 64),
                        )
                    nc.vector.tensor_mul(
                        hob[0:64, qs:qs + QT], ho_u[0:64, qs:qs + QT],
                        rbc_ps[0:64, 0:QT])
                    nc.vector.tensor_mul(
                        hob[64:128, qs:qs + QT], ho_u[64:128, qs:qs + QT],
                        rbc_ps[64:128, QT:2 * QT])
                    for dch in range(CCH):
                        op = sp.tile([128, QT], F32, tag="sc")
                        nc.tensor.matmul(
                            op[:], wo_sb[:, dch * 128:(dch + 1) * 128],
                            hob[:, qs:qs + QT], start=True, stop=True,
                        )
                        ot = osb.tile([128, QT], F16, tag="ot")
                        nc.vector.tensor_copy(ot[:], op[:])
                        nc.sync.dma_start(
                            out=outT_d[dch * 128:(dch + 1) * 128,
                                       qs:qs + QT],
                            in_=ot[:],
                        )
    nc.finalize()
    return nc


def _host_inputs(x, wq, bq, wk, bk, wv, bv, wo):
    """Per-core input maps. Slot A of core c = head c; slot B = head 8+c
    (cores 0-3) or a dummy zero head (cores 4-7)."""
    bf16 = ml_dtypes.bfloat16
    # block-major xTb: [128, tt*3072 + j*512 + i] = x[tt*512+i, j*128+p]
    xt = x[0].reshape(NQT, QT, CCH, 128)          # [tt, i, j, p]
    xTb = np.ascontiguousarray(
        xt.transpose(3, 0, 2, 1).reshape(128, NQT * BLK)).astype(bf16)
    masks = np.zeros((4, 128, QT), np.float32)
    dk = np.arange(128)[:, None]
    dq = np.arange(QT)[None, :]
    for p in range(4):
        masks[p] = (dk + 128 * p <= dq)
    masks2 = np.ascontiguousarray(
        masks.transpose(1, 0, 2).reshape(128, 4 * QT)).astype(bf16)

    in_maps = []
    for c in range(N_CORES):
        hA = c
        hB = 8 + c if c < 4 else None
        w = np.zeros((3, D_MODEL, 128), np.float32)
        b = np.zeros((128, 3), np.float32)
        bvrow = np.zeros((1, 128), np.float32)
        wo2 = np.zeros((128, D_MODEL), np.float32)
        for s, (W, B) in enumerate(((wq, bq), (wk, bk), (wv, bv))):
            w[s, :, 0:64] = W[hA]
            if s < 2:
                b[0:64, s] = B[hA]
            if hB is not None:
                w[s, :, 64:128] = W[hB]
                if s < 2:
                    b[64:128, s] = B[hB]
        bvrow[0, 0:64] = bv[hA]
        if hB is not None:
            bvrow[0, 64:128] = bv[hB]
        wo2[0:64, :] = wo[hA * 64:(hA + 1) * 64, :]
        if hB is not None:
            wo2[64:128, :] = wo[hB * 64:(hB + 1) * 64, :]
        # w2[p, (s*6+j)*128 + d] = w[s, j*128+p, d]
        w2 = np.ascontiguousarray(
            w.reshape(3, CCH, 128, 128).transpose(2, 0, 1, 3)
            .reshape(128, 3 * CCH * 128)).astype(bf16)
        in_maps.append({
            "xTb": xTb,
            "w2": w2,
            "bqkv": b.astype(np.float32),
            "bvrow": bvrow.astype(bf16),
            "wo2": wo2.astype(bf16),
            "masks2": masks2,
            "identf": np.eye(128, dtype=np.float32),
        })
    return in_maps


def kernel(_trace=False, _tmpdir=None, **inputs):
    x = np.asarray(inputs["x"], np.float32)
    args = (x,
            np.asarray(inputs["wq"], np.float32), np.asarray(inputs["bq"], np.float32),
            np.asarray(inputs["wk"], np.float32), np.asarray(inputs["bk"], np.float32),
            np.asarray(inputs["wv"], np.float32), np.asarray(inputs["bv"], np.float32),
            np.asarray(inputs["wo"], np.float32))
    bo = np.asarray(inputs["bo"], np.float32)

    if "nc" not in _PROGRAM_CACHE:
        _PROGRAM_CACHE["nc"] = build_program()
    nc = _PROGRAM_CACHE["nc"]

    in_maps = _host_inputs(*args)
    res = run_bass_kernel_spmd(
        nc, in_maps, list(range(N_CORES)), trace=_trace, tmpdir=_tmpdir,
    )
    acc = np.zeros((D_MODEL, T), np.float32)
    for c in range(N_CORES):
        acc += res.results[c]["outT"].astype(np.float32)
    out = acc.T + bo[None, :]
    if _trace:
        return out[None].astype(np.float32), res
    return out[None].astype(np.float32)


# revision 21
# speedup vs baseline: 1.0786x; 1.0525x over previous
"""Multi-head causal attention (B=1, T=4096, D=768, H=12) on 8 trn2 cores.

Sharding: 16 uniform head-slots (2 per core), 12 real heads + 4 dummy
(zero-weight) slots.  Every core runs the IDENTICAL program (SPMD); cores
differ only in the weight data they receive.  Each core computes, for its
two head-slots, the full causal attention over all 4096 tokens plus that
slot-pair's partial output projection (out.T = wo_slice.T @ headout).  The
host sums the 8 partial [768, 4096] fp16 outputs, transposes, adds bias.

Pipeline structure (v4): x arrives in 8 token blocks (block-major DRAM
layout, one 128-line DMA per block; all constants are host-packed so each
needs a single DMA).  Per block tt: project Q/K for those 512 tokens,
build V^T directly via transposed matmuls (lhsT = x chunk), then run
attention for query tile tt.  Softmax normalization is deferred and staged
across later emission points (denominator bounce -> batched reciprocal ->
partition broadcast -> scale + out-projection); the last two query tiles
get their own single-tile rounds, partially emitted between qtile-7
attention groups, to shrink the serial tail.
"""

import math
import numpy as np
import ml_dtypes
from contextlib import ExitStack

import concourse.bass as bass
import concourse.bacc as bacc
import concourse.mybir as mybir
import concourse.tile as tile
from concourse.bass_utils import run_bass_kernel_spmd

BF16 = mybir.dt.bfloat16
F16 = mybir.dt.float16
F32 = mybir.dt.float32
AF = mybir.ActivationFunctionType

T = 4096
D_MODEL = 768
HEAD_DIM = 64
N_HEADS = 12
N_CORES = 8
QT = 512                  # query tile width (one full PSUM bank per chunk)
KC = 128                  # key chunk (psum partition dim)
GRP = 3                   # score chunk-jobs per exp group -> ACT free dim 1536
NQT = T // QT             # 8 query tiles / token blocks
CCH = D_MODEL // 128      # 6 contraction chunks
BLK = QT * CCH            # 3072 cols per block in xTb layout
VST = 208                 # V2 column stride per key chunk

_PROGRAM_CACHE = {}


def build_program():
    nc = bacc.Bacc(None)

    # xTb block-major: xTb[p, tt*3072 + j*512 + i] = x[tt*512 + i, j*128 + p]
    xT_d = nc.declare_dram_parameter("xTb", [128, NQT * BLK], BF16, isOutput=False)
    # w2 pre-packed in SBUF layout: w2[p, (s*6+j)*128 + d] = W_s[j*128+p, d]
    w_d = nc.declare_dram_parameter("w2", [128, 3 * CCH * 128], BF16, isOutput=False)
    b_d = nc.declare_dram_parameter("bqkv", [128, 3], F32, isOutput=False)
    bvr_d = nc.declare_dram_parameter("bvrow", [1, 128], BF16, isOutput=False)
    wo_d = nc.declare_dram_parameter("wo2", [128, D_MODEL], BF16, isOutput=False)
    mk_d = nc.declare_dram_parameter("masks2", [128, 4 * QT], BF16, isOutput=False)
    id_d = nc.declare_dram_parameter("identf", [128, 128], F32, isOutput=False)
    outT_d = nc.declare_dram_parameter("outT", [D_MODEL, T], F16, isOutput=True)

    with tile.TileContext(nc) as tc, ExitStack() as ctx:
        consts = ctx.enter_context(tc.tile_pool(name="consts", bufs=1))
        big = ctx.enter_context(tc.tile_pool(name="big", bufs=1))
        ptp = ctx.enter_context(tc.tile_pool(name="ptp", bufs=4))
        rp = ctx.enter_context(tc.tile_pool(name="rp", bufs=3))
        osb = ctx.enter_context(tc.tile_pool(name="osb", bufs=6))
        sp = ctx.enter_context(tc.tile_pool(name="sp", bufs=2, space="PSUM"))
        avp = ctx.enter_context(tc.tile_pool(name="avp", bufs=1, space="PSUM"))
        dramp = ctx.enter_context(tc.tile_pool(name="dramp", bufs=2, space="DRAM"))

        xT_sb = big.tile([128, NQT * BLK], BF16, tag="xTb")
        w_sb = consts.tile([128, 3 * CCH * 128], BF16, tag="w")
        b_sb = consts.tile([128, 3], F32, tag="b")
        bvr_sb = consts.tile([1, 128], BF16, tag="bvr")
        wo_sb = consts.tile([128, D_MODEL], BF16, tag="wo")
        mask_sb = consts.tile([128, 4 * QT], BF16, tag="mask")
        ones_sb = consts.tile([1, QT], BF16, tag="ones1")
        id_sb = consts.tile([128, 128], F32, tag="identf")

        # w + b + first x block first so projections start ASAP
        nc.sync.dma_start(out=w_sb[:], in_=w_d[:, :])
        nc.sync.dma_start(out=b_sb[:], in_=b_d[:, :])
        nc.sync.dma_start(out=xT_sb[:, 0:BLK], in_=xT_d[:, 0:BLK])
        nc.sync.dma_start(out=wo_sb[:], in_=wo_d[:, :])
        nc.sync.dma_start(out=mask_sb[:], in_=mk_d[:, :])
        nc.sync.dma_start(out=bvr_sb[:], in_=bvr_d[:, :])
        nc.sync.dma_start(out=id_sb[:], in_=id_d[:, :])
        nc.vector.memset(ones_sb[:], 1.0)

        # warm the PE (HAM clock gate) during the input-DMA window with
        # throwaway rank-1 matmuls; results land in a psum tile nobody reads
        warm = avp.tile([128, 2 * QT], F32, tag="av")
        for _ in range(28):
            nc.tensor.matmul(
                warm[0:128, 0:QT], ones_sb[:, 0:128], ones_sb[:, 0:QT],
                start=True, stop=True,
            )

        QT_sb = big.tile([128, T], BF16, tag="qsb")
        KT_sb = big.tile([128, T], BF16, tag="ksb")
        V2 = big.tile([128, (T // KC) * VST], BF16, tag="V2")
        nc.vector.memset(V2[:], 0.0)
        v3 = V2[:].rearrange("p (t c) -> p t c", c=VST)
        nc.vector.memset(v3[:, :, 64:65], 1.0)
        nc.vector.memset(v3[:, :, 97:98], 1.0)
        ho_u = big.tile([128, T], F32, tag="ho_u")      # unnormalized AV
        hob = big.tile([128, T], BF16, tag="hob")       # normalized, bf16
        dn_st = big.tile([128, 2 * QT], F32, tag="dn")  # denom staging
        nc.vector.memset(dn_st[:], 0.0)

        rounds = {}  # round key -> state

        def xchunk(tt, j, lo, hi):
            base = tt * BLK + j * QT
            return xT_sb[:, base + lo:base + hi]

        def stage_a(qts):
            # denominator rows (dA row 64 bank0, dB row 32 bank1, already
            # staged in dn_st) -> DRAM rd -> partition-spread dn_sb so one
            # [128, 16] reciprocal covers up to 2 qtiles x 2 heads
            n = len(qts)
            c0 = (qts[0] % 2) * QT
            rd = dramp.tile([2, 2 * QT], F32, tag="rd")
            nc.sync.dma_start(out=rd[0:1, 0:n * QT],
                              in_=dn_st[64:65, c0:c0 + n * QT])
            nc.sync.dma_start(out=rd[1:2, 0:n * QT],
                              in_=dn_st[32:33, c0:c0 + n * QT])
            dn_sb = rp.tile([128, 16], F32, tag="dnsb")
            for r in range(2):  # rd row r at linear offset r*2*QT
                nc.sync.dma_start(
                    out=dn_sb[:, r * 8:r * 8 + 4 * n],
                    in_=bass.AP(tensor=rd.tensor, offset=rd.offset + r * 2 * QT,
                                ap=[[4 * n, 128], [1, 4 * n]]))
            rounds[tuple(qts)] = {"dn_sb": dn_sb}

        def stage_b(qts):
            n = len(qts)
            st = rounds[tuple(qts)]
            dn_r = rp.tile([128, 16], F32, tag="dnr")
            nc.vector.reciprocal(dn_r[:, 0:16], st["dn_sb"][:, 0:16])
            rr = dramp.tile([1, 4 * QT], F32, tag="rr")
            for r in range(2):
                nc.sync.dma_start(
                    out=bass.AP(tensor=rr.tensor, offset=rr.offset + r * n * QT,
                                ap=[[4 * n, 128], [1, 4 * n]]),
                    in_=dn_r[:, r * 8:r * 8 + 4 * n])
            # rr linear: [rA(qts[0]) .. rA(qts[n-1]) | rB(qts[0]) .. ]
            rbcs = []
            for i in range(n):
                rbc = rp.tile([128, QT], F32, tag="rbc")
                rA = rr[0:1, i * QT:(i + 1) * QT]
                rB = rr[0:1, (n + i) * QT:(n + i + 1) * QT]
                nc.gpsimd.dma_start(
                    out=rbc[0:64, :],
                    in_=bass.AP(tensor=rA.tensor, offset=rA.offset,
                                ap=[[0, 64]] + list(rA.ap[1:])))
                nc.gpsimd.dma_start(
                    out=rbc[64:128, :],
                    in_=bass.AP(tensor=rB.tensor, offset=rB.offset,
                                ap=[[0, 64]] + list(rB.ap[1:])))
                rbcs.append(rbc)
            st["rbcs"] = rbcs

        def stage_c(qts, only=None, pop=True):
            st = rounds[tuple(qts)]
            if pop:
                rounds.pop(tuple(qts))
            for i, q2 in enumerate(qts):
                if only is not None and i != only:
                    continue
                q2s = q2 * QT
                nc.vector.tensor_mul(
                    hob[:, q2s:q2s + QT], ho_u[:, q2s:q2s + QT],
                    st["rbcs"][i][:])
                for dch in range(CCH):
                    op = sp.tile([128, QT], F32, tag="sc")
                    nc.tensor.matmul(
                        op[:], wo_sb[:, dch * 128:(dch + 1) * 128],
                        hob[:, q2s:q2s + QT], start=True, stop=True,
                    )
                    ot = osb.tile([128, QT], F16, tag="ot")
                    nc.vector.tensor_copy(ot[:], op[:])
                    nc.sync.dma_start(
                        out=outT_d[dch * 128:(dch + 1) * 128, q2s:q2s + QT],
                        in_=ot[:],
                    )

        def proj_unit(tt, s):
            dst = QT_sb if s == 0 else KT_sb
            with nc.named_scope("proj"):
                pp = sp.tile([128, QT], F32, tag="sc")
                for j in range(CCH):
                    nc.tensor.matmul(
                        pp[:],
                        w_sb[:, (s * CCH + j) * 128:(s * CCH + j + 1) * 128],
                        xchunk(tt, j, 0, QT),
                        start=(j == 0), stop=(j == CCH - 1),
                    )
                nc.vector.tensor_scalar_add(
                    dst[:, tt * QT:(tt + 1) * QT], pp[:], b_sb[:, s:s + 1])

        def vt_unit(tt, q4):
            with nc.named_scope("vt"):
                tt4 = tt * 4 + q4
                vt = sp.tile([128, 128], F32, tag="sc")
                for j in range(CCH):
                    base = (2 * CCH + j) * 128
                    nc.tensor.matmul(
                        vt[:], xchunk(tt, j, q4 * KC, (q4 + 1) * KC),
                        w_sb[:, base:base + 128],
                        start=(j == 0), stop=False,
                    )
                nc.tensor.matmul(  # rank-1 bias: out[tok, :] += bv
                    vt[:], ones_sb[:, 0:128], bvr_sb[:, :],
                    start=False, stop=True,
                )
                nc.vector.tensor_copy(
                    V2[:, tt4 * VST:tt4 * VST + 64], vt[:, 0:64])
                nc.vector.tensor_copy(
                    V2[:, tt4 * VST + 129:tt4 * VST + 193], vt[:, 64:128])

        # ================= main token-block pipeline =================
        # block 0 work runs up front; block tt+1 proj/V^T are emitted as
        # fillers between attention groups of qtile tt (the early qtiles are
        # ACT-limited, so the PE absorbs them for free)
        for s in range(2):
            proj_unit(0, s)
        for q4 in range(4):
            vt_unit(0, q4)
        for tt in range(NQT):
            if tt + 1 < NQT:
                nc.sync.dma_start(
                    out=xT_sb[:, (tt + 1) * BLK:(tt + 2) * BLK],
                    in_=xT_d[:, (tt + 1) * BLK:(tt + 2) * BLK])

            # ---- attention for query tile qi = tt ----
            qi = tt
            qs = qi * QT
            # av bank 0: head-A group (AV rows 0:64, denom row 64)
            # av bank 1: head-B group (denom row 32, AV rows 64:128)
            av = avp.tile([128, 2 * QT], F32, tag="av")
            nsteps = 4 * (qi + 1)
            jobs = [(kc, h) for kc in range(nsteps) for h in (0, 1)]
            # stage work injected between qtile-7 attention groups
            mid = {}
            if qi == 7:
                mid = {4: lambda: stage_b((4, 5)), 8: lambda: stage_b((6,)),
                       10: lambda: stage_c((4, 5), only=0, pop=False),
                       14: lambda: stage_c((4, 5), only=1),
                       18: lambda: stage_c((6,))}
            def emit_scores(grp):
                width = len(grp) * QT
                with nc.named_scope("score"):
                    sc = sp.tile([128, GRP * QT], F32, tag="sc")
                    for ji, (kc, h) in enumerate(grp):
                        nc.tensor.matmul(
                            sc[:, ji * QT:(ji + 1) * QT],
                            KT_sb[64 * h:64 * h + 64, kc * KC:(kc + 1) * KC],
                            QT_sb[64 * h:64 * h + 64, qs:qs + QT],
                            start=True, stop=True, tile_position=(64 * h, 0),
                        )
                pt = ptp.tile([128, GRP * QT], BF16, tag="pt")
                with nc.named_scope("exp"):
                    nc.scalar.activation(
                        pt[:, :width], sc[:, :width], AF.Exp,
                        scale=1.0 / math.sqrt(HEAD_DIM),
                    )
                with nc.named_scope("av"):
                    for ji, (kc, h) in enumerate(grp):
                        if kc >= 4 * qi:  # diagonal straddle
                            ptj = pt[:, ji * QT:(ji + 1) * QT]
                            pat = kc - 4 * qi
                            m = mask_sb[:, pat * QT:(pat + 1) * QT]
                            nc.vector.tensor_mul(ptj, ptj, m)
                return grp, pt

            def emit_av(grp, pt):
                with nc.named_scope("av"):
                    for ji, (kc, h) in enumerate(grp):
                        ptj = pt[:, ji * QT:(ji + 1) * QT]
                        st_ = kc == 0
                        sp_ = kc == nsteps - 1
                        vbase = kc * VST
                        if h == 0:
                            # lhsT padded to 128 cols; rows 65:128 junk
                            nc.tensor.matmul(
                                av[0:128, 0:QT], V2[:, vbase:vbase + 128], ptj,
                                start=st_, stop=sp_, tile_position=(0, 0),
                            )
                        else:
                            nc.tensor.matmul(
                                av[0:128, QT:2 * QT],
                                V2[:, vbase + 65:vbase + 193],
                                ptj, start=st_, stop=sp_, tile_position=(0, 0),
                            )

            # score groups emitted in adjacent pairs (6 back-to-back score
            # matmuls -> every 64-row half pairs with a concurrent partner),
            # with each pair's AV matmuls deferred until after the NEXT
            # pair's scores, so AV never waits on the exp of its own group
            pending = []
            for gn, g in enumerate(range(0, len(jobs), GRP)):
                if gn in mid:
                    mid[gn]()
                if gn % 2 == 0 and len(pending) == 2:
                    for item in pending:
                        emit_av(*item)
                    pending = []
                pending.append(emit_scores(jobs[g:g + GRP]))
            for item in pending:
                emit_av(*item)
            if tt + 1 < NQT:
                for s in range(2):
                    proj_unit(tt + 1, s)
                for q4 in range(4):
                    vt_unit(tt + 1, q4)
            # stash unnormalized AV (DVE) + denominators (ScalarE, parallel)
            with nc.named_scope("stash"):
                dcol = (qi % 2) * QT
                nc.scalar.copy(dn_st[64:65, dcol:dcol + QT], av[64:65, 0:QT])
                nc.scalar.copy(dn_st[32:33, dcol:dcol + QT],
                               av[32:33, QT:2 * QT])
                nc.vector.tensor_copy(ho_u[0:64, qs:qs + QT], av[0:64, 0:QT])
                nc.vector.tensor_copy(
                    ho_u[64:128, qs:qs + QT], av[64:128, QT:2 * QT])

            # staged deferred normalization + out-projection
            with nc.named_scope("norm"):
                if qi == 1:
                    stage_a((0, 1))
                elif qi == 2:
                    stage_b((0, 1))
                    stage_c((0, 1), only=0, pop=False)
                elif qi == 3:
                    stage_a((2, 3))
                    stage_c((0, 1), only=1)
                elif qi == 4:
                    stage_b((2, 3))
                    stage_c((2, 3), only=0, pop=False)
                elif qi == 5:
                    stage_a((4, 5))
                    stage_c((2, 3), only=1)
                elif qi == 6:
                    stage_a((6,))
                elif qi == 7:
                    # on-chip final round: PE transposes spread the
                    # denominators across partitions, one [128, 8]
                    # reciprocal, column transposes bring the recips back
                    # to partition 0, rank-1 matmuls broadcast them; no
                    # DRAM bounce in the serial tail
                    tp = sp.tile([128, GRP * QT], F32, tag="sc")
                    for k in range(4):
                        nc.tensor.transpose(
                            tp[:, k * 128:(k + 1) * 128],
                            dn_st[:, QT + k * 128:QT + (k + 1) * 128],
                            id_sb[:])
                    r8 = rp.tile([128, 16], F32, tag="dnsb")
                    selA = tp[:, 64:65]
                    selB = tp[:, 32:33]
                    nc.vector.reciprocal(
                        r8[:, 0:4],
                        bass.AP(tensor=selA.tensor, offset=selA.offset,
                                ap=[list(selA.ap[0]), [128, 4]]))
                    nc.vector.reciprocal(
                        r8[:, 4:8],
                        bass.AP(tensor=selB.tensor, offset=selB.offset,
                                ap=[list(selB.ap[0]), [128, 4]]))
                    for k in range(8):
                        nc.tensor.transpose(
                            tp[0:1, QT + k * 128:QT + (k + 1) * 128],
                            r8[:, k:k + 1], id_sb[:])
                    rr_row = rp.tile([1, 2 * QT], BF16, tag="rrow")
                    nc.vector.tensor_copy(
                        rr_row[0:1, :], tp[0:1, QT:QT + 2 * QT])
                    rbc_ps = sp.tile([128, 2 * QT], F32, tag="sc")
                    for k in range(4):
                        nc.tensor.matmul(
                            rbc_ps[0:64, k * 128:(k + 1) * 128],
                            ones_sb[0:1, 0:64],
                            rr_row[0:1, k * 128:(k + 1) * 128],
                            start=True, stop=True, tile_position=(0, 0),
                        )
                        nc.tensor.matmul(
                            rbc_ps[64:128, QT + k * 128:QT + (k + 1) * 128],
                            ones_sb[0:1, 0:64],
                            rr_row[0:1, (4 + k) * 128:(5 + k) * 128],
                            start=True, stop=True, tile_position=(0, 64),
                        )
                    nc.vector.tensor_mul(
                        hob[0:64, qs:qs + QT], ho_u[0:64, qs:qs + QT],
                        rbc_ps[0:64, 0:QT])
                    nc.vector.tensor_mul(
                        hob[64:128, qs:qs + QT], ho_u[64:128, qs:qs + QT],
                        rbc_ps[64:128, QT:2 * QT])
                    for dch in range(CCH):
                        op = sp.tile([128, QT], F32, tag="sc")
                        nc.tensor.matmul(
                            op[:], wo_sb[:, dch * 128:(dch + 1) * 128],
                            hob[:, qs:qs + QT], start=True, stop=True,
                        )
                        ot = osb.tile([128, QT], F16, tag="ot")
                        nc.vector.tensor_copy(ot[:], op[:])
                        nc.sync.dma_start(
                            out=outT_d[dch * 128:(dch + 1) * 128,
                                       qs:qs + QT],
                            in_=ot[:],
                        )
    nc.finalize()
    return nc


def _host_inputs(x, wq, bq, wk, bk, wv, bv, wo):
    """Per-core input maps. Slot A of core c = head c; slot B = head 8+c
    (cores 0-3) or a dummy zero head (cores 4-7)."""
    bf16 = ml_dtypes.bfloat16
    # block-major xTb: [128, tt*3072 + j*512 + i] = x[tt*512+i, j*128+p]
    xt = x[0].reshape(NQT, QT, CCH, 128)          # [tt, i, j, p]
    xTb = np.ascontiguousarray(
        xt.transpose(3, 0, 2, 1).reshape(128, NQT * BLK)).astype(bf16)
    masks = np.zeros((4, 128, QT), np.float32)
    dk = np.arange(128)[:, None]
    dq = np.arange(QT)[None, :]
    for p in range(4):
        masks[p] = (dk + 128 * p <= dq)
    masks2 = np.ascontiguousarray(
        masks.transpose(1, 0, 2).reshape(128, 4 * QT)).astype(bf16)

    in_maps = []
    for c in range(N_CORES):
        hA = c
        hB = 8 + c if c < 4 else None
        w = np.zeros((3, D_MODEL, 128), np.float32)
        b = np.zeros((128, 3), np.float32)
        bvrow = np.zeros((1, 128), np.float32)
        wo2 = np.zeros((128, D_MODEL), np.float32)
        for s, (W, B) in enumerate(((wq, bq), (wk, bk), (wv, bv))):
            w[s, :, 0:64] = W[hA]
            if s < 2:
                b[0:64, s] = B[hA]
            if hB is not None:
                w[s, :, 64:128] = W[hB]
                if s < 2:
                    b[64:128, s] = B[hB]
        bvrow[0, 0:64] = bv[hA]
        if hB is not None:
            bvrow[0, 64:128] = bv[hB]
        wo2[0:64, :] = wo[hA * 64:(hA + 1) * 64, :]
        if hB is not None:
            wo2[64:128, :] = wo[hB * 64:(hB + 1) * 64, :]
        # w2[p, (s*6+j)*128 + d] = w[s, j*128+p, d]
        w2 = np.ascontiguousarray(
            w.reshape(3, CCH, 128, 128).transpose(2, 0, 1, 3)
            .reshape(128, 3 * CCH * 128)).astype(bf16)
        in_maps.append({
            "xTb": xTb,
            "w2": w2,
            "bqkv": b.astype(np.float32),
            "bvrow": bvrow.astype(bf16),
            "wo2": wo2.astype(bf16),
            "masks2": masks2,
            "identf": np.eye(128, dtype=np.float32),
        })
    return in_maps


def kernel(_trace=False, _tmpdir=None, **inputs):
    x = np.asarray(inputs["x"], np.float32)
    args = (x,
            np.asarray(inputs["wq"], np.float32), np.asarray(inputs["bq"], np.float32),
            np.asarray(inputs["wk"], np.float32), np.asarray(inputs["bk"], np.float32),
            np.asarray(inputs["wv"], np.float32), np.asarray(inputs["bv"], np.float32),
            np.asarray(inputs["wo"], np.float32))
    bo = np.asarray(inputs["bo"], np.float32)

    if "nc" not in _PROGRAM_CACHE:
        _PROGRAM_CACHE["nc"] = build_program()
    nc = _PROGRAM_CACHE["nc"]

    in_maps = _host_inputs(*args)
    res = run_bass_kernel_spmd(
        nc, in_maps, list(range(N_CORES)), trace=_trace, tmpdir=_tmpdir,
    )
    acc = np.zeros((D_MODEL, T), np.float32)
    for c in range(N_CORES):
        acc += res.results[c]["outT"].astype(np.float32)
    out = acc.T + bo[None, :]
    if _trace:
        return out[None].astype(np.float32), res
    return out[None].astype(np.float32)
